# revision 44
# baseline (speedup 1.0000x reference)
"""Trainium2 Bass kernel for nn_CausalSelfAttention_67894843015857.

Full inputs -> full output. Sharding: 8 cores = 2 batches x 4 head-groups
(4 heads each). Per core, on device:
  - q,k projections (W stationary, x^T moving) -> qT/kT in [dim, t] layout
  - RoPE (host-precomputed cos/sin tables from token_index histogram cumsum)
    + fork-channel overwrites, on-chip
  - v projection (x^T stationary) -> V in [t, dim] layout, scaled by
    exp(cumulative_scores)*padmask, with a ones-column appended (softmax denom)
  - attention with TRANSPOSED scores S^T[tk, tq] (no P-transpose needed),
    no-max-subtraction softmax (scores are bounded ~2.6), causal masking via
    0/1 masks on the 4 diagonal blocks of each 512-wide tq chunk
  - output projection -> per-core partial [T, C]
Host: reduces the 4 per-batch partials and adds b_proj.

All matmul paths use float32r (TF32-class) end-to-end: DMA/ACT/DVE producers
write f32r so the BIR verifier accepts the f32r matmuls (1 cycle/row).
"""
import numpy as np

import concourse.bacc as bacc
import concourse.mybir as mybir
import concourse.tile as tile
from concourse.bass_utils import run_bass_kernel_spmd

F32 = mybir.dt.float32
F32R = mybir.dt.float32r
AF = mybir.ActivationFunctionType

P = 128
T = 2048
C = 1024
NKT = C // P          # 8 contraction tiles over the embedding dim
NT = T // P           # 16 t-tiles
SCALE = 0.125         # 1/sqrt(64)
NCHUNK = 4            # tq chunks of 512
CH = 512

_NC_CACHE = {}


def build_nc():
    if "nc" in _NC_CACHE:
        return _NC_CACHE["nc"]
    nc = bacc.Bacc("TRN2", target_bir_lowering=False, debug=False)

    def din(name, shape, dt=F32R):
        return nc.dram_tensor(name, shape, dt, kind="ExternalInput").ap()

    xt_d = din("xt", [C, T])            # x[b].T
    wqk_d = din("wqk", [C, 512])        # [q cols 256 | k cols 256] for this head group
    wv_d = din("wv", [C, 256])
    wp_d = din("wp", [256, C])
    bqk_d = din("bqk", [P, 8], F32)     # cols 0-3: bias per col-tile; 4-7: rotated bias
    bv_d = din("bv", [1, 256])
    cos_d = din("cos2", [P, T])         # cos table, 4x stacked 32-row groups
    nsin_d = din("nsin", [P, T])        # [-sin, +sin, -sin, +sin] row groups (dest-indexed)
    cs_d = din("csrow", [1, T])         # cumulative_scores[b]
    vs_d = din("vscale", [P, 16], F32)       # exp(cs)*pmbin, t-tiled columns
    oc_d = din("onec", [P, 16])         # pmbin, t-tiled columns
    dm_d = din("dmask", [P, P])  # upper-tri 0/1 mask: m[p,c] = c >= p
    ones_d = din("ones", [1, T])
    out_d = nc.dram_tensor("outp", [T, C], F32, kind="ExternalOutput").ap()

    with tile.TileContext(nc) as tc:
        with tc.tile_pool(name="const", bufs=1) as pc, \
             tc.tile_pool(name="persist", bufs=1) as pp:
            bqk_sb = pc.tile([P, 8], F32, name="bqk_sb")
            nc.sync.dma_start(bqk_sb[:], bqk_d[:])
            bv_sb = pc.tile([1, 256], F32R, name="bv_sb")
            nc.sync.dma_start(bv_sb[:], bv_d[:])
            ones_sb = pc.tile([1, P], F32R, name="ones_sb")
            nc.sync.dma_start(ones_sb[:], ones_d[0:1, 0:P])
            vs_sb = pc.tile([P, 16], F32, name="vs_sb")
            nc.sync.dma_start(vs_sb[:], vs_d[:])
            oc_sb = pc.tile([P, 16], F32R, name="oc_sb")
            nc.sync.dma_start(oc_sb[:], oc_d[:])
            dm_sb = pc.tile([P, P], F32R, name="dm_sb")
            nc.sync.dma_start(dm_sb[:], dm_d[:])

            qk_t = [pp.tile([P, T], F32R, name=f"qkt{m}") for m in range(4)]
            vv = pp.tile([P, NT * 260], F32R, name="vv")
            yt = [pp.tile([P, T], F32R, name=f"yt{i}") for i in range(2)]

            pat = tc.alloc_tile_pool(name="attn_sb", bufs=5)
            pnm = tc.alloc_tile_pool(name="norm_sb", bufs=2)
            pmid = tc.alloc_tile_pool(name="mid", bufs=1)
            with tc.tile_pool(name="load", bufs=1) as pl, \
                 tc.tile_pool(name="qkps", bufs=1, space="PSUM") as qkps, \
                 tc.tile_pool(name="vps", bufs=4, space="PSUM") as vpsp, \
                 tc.tile_pool(name="ropetmp", bufs=1) as ptmp_unused:
                xt = pl.tile([P, NKT * T], F32R, name="xt_sb")
                wqk = pl.tile([P, NKT * 512], F32R, name="wqk_sb")
                for k in range(NKT):
                    nc.sync.dma_start(xt[:, k * T: k * T + CH],
                                      xt_d[k * P:(k + 1) * P, 0:CH])
                    nc.sync.dma_start(wqk[:, k * 512:(k + 1) * 512], wqk_d[k * P:(k + 1) * P, :])
                    for n in range(1, 4):
                        nc.sync.dma_start(xt[:, k * T + n * CH: k * T + (n + 1) * CH],
                                          xt_d[k * P:(k + 1) * P, n * CH:(n + 1) * CH])
                cos_sb = pmid.tile([P, T], F32R, name="cos_sb")
                nc.sync.dma_start(cos_sb[:], cos_d[:])
                nsin_sb = pmid.tile([P, T], F32R, name="nsin_sb")
                nc.sync.dma_start(nsin_sb[:], nsin_d[:])
                wv = pl.tile([P, NKT * 256], F32R, name="wv_sb")
                for k in range(NKT):
                    nc.sync.dma_start(wv[:, k * 256:(k + 1) * 256], wv_d[k * P:(k + 1) * P, :])

                def qk_mm(m):
                    """q,k projection matmuls + psum->sbuf bias copies."""
                    pss = [qkps.tile([P, CH], F32, name=f"qkps{m}_{n}", tag=f"qk{n}")
                           for n in range(4)]
                    for k in range(NKT):
                        for n in range(4):
                            nc.tensor.matmul(
                                pss[n][:],
                                lhsT=wqk[:, k * 512 + m * P: k * 512 + (m + 1) * P],
                                rhs=xt[:, k * T + n * CH: k * T + (n + 1) * CH],
                                start=(k == 0), stop=(k == NKT - 1))
                    for n in range(4):
                        ns = slice(n * CH, (n + 1) * CH)
                        nc.scalar.activation(qk_t[m][:, ns], pss[n][:], AF.Identity,
                                             bias=bqk_sb[:, m:m + 1])

                def rope_group(m):
                    """In-place RoPE on qk_t[m] via a DMA-built partition-rotated
                    copy (qsh), then fork-channel overwrites."""
                    qsh = pmid.tile([P, T], F32R, name=f"qsh{m}", tag="qsh")
                    for n in range(4):
                        ns = slice(n * CH, (n + 1) * CH)
                        for blk in range(4):
                            sr = 32 * (blk + 1) if blk % 2 == 0 else 32 * (blk - 1)
                            nc.sync.dma_start(qsh[32 * blk:32 * (blk + 1), ns],
                                              qk_t[m][sr:sr + 32, ns])
                        # in-place RoPE: q = q*cos + qsh*nsin
                        nc.vector.tensor_mul(qk_t[m][:, ns], qk_t[m][:, ns],
                                             cos_sb[:, ns])
                        nc.vector.tensor_mul(qsh[:, ns], qsh[:, ns], nsin_sb[:, ns])
                        nc.vector.tensor_add(qk_t[m][:, ns], qk_t[m][:, ns],
                                             qsh[:, ns])
                        # fork-channel overwrites for this chunk
                        src = ones_d if m < 2 else cs_d
                        for row in (63, 127):
                            nc.sync.dma_start(qk_t[m][row:row + 1, ns],
                                              src[0:1, ns])

                def qk_group(m):
                    qk_mm(m)
                    rope_group(m)

                def v_group(mt0, mt1):
                    """v projection: out[t, vcol] = xT_tile.T @ wv; build V''."""
                    for mt in range(mt0, mt1):
                        vps = vpsp.tile([P, 256], F32, name=f"vps{mt}", tag="vps")
                        for k in range(NKT):
                            nc.tensor.matmul(
                                vps[:],
                                lhsT=xt[:, k * T + mt * P: k * T + (mt + 1) * P],
                                rhs=wv[:, k * 256:(k + 1) * 256],
                                start=(k == 0), stop=False)
                        nc.tensor.matmul(vps[:], lhsT=ones_sb[0:1, 0:P],
                                         rhs=bv_sb[0:1, :], start=False, stop=True)
                        vvs = vv[:, mt * 260:(mt + 1) * 260].rearrange(
                            "p (h x) -> p h x", x=65)
                        nc.vector.tensor_scalar_mul(
                            vvs[:, :, 0:64],
                            vps[:].rearrange("p (h x) -> p h x", x=64),
                            vs_sb[:, mt:mt + 1])
                        nc.vector.tensor_copy(
                            vvs[:, :, 64:65],
                            oc_sb[:, mt:mt + 1, None].to_broadcast((P, 4, 1)))

                def norm_chunk(h, cch, yps, pyps, psps):
                    # normalize: y = num / den  (den = ones-column row 64)
                    ti = h // 2
                    ro = 64 * (h % 2)
                    recip = pnm.tile([1, CH], F32R, name=f"rc_{h}_{cch}", tag="rc")
                    with nc.allow_low_precision(reason="f32r recip feeds f32r bcast matmul"):
                        nc.vector.reciprocal(recip[0:1, :], yps[64:65, :])
                    # broadcast along partitions via K=1 ones matmul
                    bps = pyps.tile([64, CH], F32, name=f"bp_{h}_{cch}", tag="bps")
                    nc.tensor.matmul(bps[:], lhsT=ones_sb[0:1, 0:64],
                                     rhs=recip[0:1, :], start=True, stop=True)
                    bsb = pnm.tile([64, CH], F32, name=f"bs_{h}_{cch}", tag="bs")
                    nc.vector.tensor_copy(bsb[:], bps[:])
                    nc.vector.tensor_mul(
                        yt[ti][ro:ro + 64, cch * CH:(cch + 1) * CH],
                        yps[0:64, :], bsb[:])

                norm_state = {"pending": None}

                def attn_chunks(h, c0, c1, psps, pyps):
                    ti = h // 2
                    ro = 64 * (h % 2)
                    qt = qk_t[ti]
                    kt = qk_t[2 + ti]
                    for cch in range(c0, c1):
                        nik = 4 * (cch + 1)
                        yps = pyps.tile([65, CH], F32, name=f"yps_{h}_{cch}",
                                        tag="yps")
                        for p2 in range(nik // 2):
                            # an ik pair shares one 2-bank PSUM tile so full
                            # pairs need only a single wide exp
                            spw = psps.tile([P, 2 * CH], F32,
                                            name=f"spw_{h}_{cch}_{p2}", tag="sps")
                            pt = pat.tile([P, 2 * CH], F32R,
                                          name=f"pt_{h}_{cch}_{p2}", tag="pt")
                            iks = (2 * p2, 2 * p2 + 1)
                            los = [max(ik - 4 * cch, 0) * P for ik in iks]
                            for ii, ik in enumerate(iks):
                                lo = los[ii]
                                nc.tensor.matmul(
                                    spw[:, ii * CH + lo:(ii + 1) * CH],
                                    lhsT=kt[ro:ro + 64, ik * P:(ik + 1) * P],
                                    rhs=qt[ro:ro + 64, cch * CH + lo:(cch + 1) * CH],
                                    start=True, stop=True)
                            if los[0] == 0 and los[1] == 0:
                                nc.scalar.activation(pt[:], spw[:], AF.Exp,
                                                     scale=SCALE)
                            else:
                                for ii, ik in enumerate(iks):
                                    lo = los[ii]
                                    nc.scalar.activation(
                                        pt[:, ii * CH + lo:(ii + 1) * CH],
                                        spw[:, ii * CH + lo:(ii + 1) * CH],
                                        AF.Exp, scale=SCALE)
                            for ii, ik in enumerate(iks):
                                lo = los[ii]
                                if ik - 4 * cch >= 0:
                                    # triangular mask on the diagonal block
                                    nc.vector.tensor_mul(
                                        pt[:, ii * CH + lo: ii * CH + lo + P],
                                        pt[:, ii * CH + lo: ii * CH + lo + P],
                                        dm_sb[:])
                                nc.tensor.matmul(
                                    yps[:, lo:CH],
                                    lhsT=vv[:, ik * 260 + h * 65: ik * 260 + h * 65 + 65],
                                    rhs=pt[:, ii * CH + lo:(ii + 1) * CH],
                                    start=(ik == 0), stop=(ik == nik - 1))
                            if p2 == 0 and norm_state["pending"] is not None:
                                # previous chunk's norm, deep in this chunk's
                                # pipeline so it never stalls PE/ACT
                                norm_chunk(*norm_state["pending"], pyps, psps)
                                norm_state["pending"] = None
                        norm_state["pending"] = (h, cch, yps)

                with nc.named_scope("qk_proj"):
                    qk_group(0)   # q heads 0,1
                    qk_group(2)   # k heads 0,1
                with nc.named_scope("v_proj"):
                    v_group(0, 16)
                with nc.named_scope("qk_proj2"):
                    qk_mm(1)      # q heads 2,3 (RoPE deferred into attn scope)
                    qk_mm(3)      # k heads 2,3

            # load pool released: xt/wqk/wv/cos/nsin space reusable
            with nc.named_scope("attn"), \
                 tc.tile_pool(name="sps", bufs=2, space="PSUM") as psps, \
                 tc.tile_pool(name="yps", bufs=2, space="PSUM") as pyps:
                attn_chunks(0, 0, 4, psps, pyps)
                # deferred RoPE for heads 2,3 -- runs on DVE under h0/h1's ACT
                rope_group(1)
                rope_group(3)
                attn_chunks(1, 0, 4, psps, pyps)
                attn_chunks(2, 0, 4, psps, pyps)
                attn_chunks(3, 0, 4, psps, pyps)
                norm_chunk(*norm_state["pending"], pyps, psps)

            with tc.tile_pool(name="late", bufs=1) as plate:
                wp_sb = plate.tile([P, 2 * C], F32R, name="wp_sb")
                for kk in range(2):
                    nc.sync.dma_start(wp_sb[:, kk * C:(kk + 1) * C],
                                      wp_d[kk * P:(kk + 1) * P, :])

                with nc.named_scope("proj"), \
                     tc.tile_pool(name="pps", bufs=4, space="PSUM") as pjps, \
                     tc.tile_pool(name="out_sb", bufs=3) as pout:
                    for mt in range(NT):
                        pps = [pjps.tile([P, CH], F32, name=f"pps{mt}_{n}", tag=f"pp{n}")
                               for n in range(2)]
                        for kk in range(2):
                            for n in range(2):
                                nc.tensor.matmul(
                                    pps[n][:],
                                    lhsT=yt[kk][:, mt * P:(mt + 1) * P],
                                    rhs=wp_sb[:, kk * C + n * CH: kk * C + (n + 1) * CH],
                                    start=(kk == 0), stop=(kk == 1))
                        osb = pout.tile([P, C], F32, name=f"osb{mt}", tag="osb")
                        nc.scalar.copy(osb[:, 0:CH], pps[0][:])
                        nc.vector.tensor_copy(osb[:, CH:C], pps[1][:])
                        nc.sync.dma_start(out_d[mt * P:(mt + 1) * P, :], osb[:])

            pmid.release()
            pnm.release()
            pat.release()
    nc.compile()
    _NC_CACHE["nc"] = nc
    return nc


def make_in_maps(x, cumulative_scores, padding_mask, W_attn, b_attn, W_proj,
                 b_proj, token_index):
    x = np.asarray(x, dtype=np.float32)
    cs = np.asarray(cumulative_scores, dtype=np.float32)
    pm = np.asarray(padding_mask, dtype=np.float32)
    Wa = np.asarray(W_attn, dtype=np.float32)
    ba = np.asarray(b_attn, dtype=np.float32)
    Wp = np.asarray(W_proj, dtype=np.float32)
    tok = np.asarray(token_index).astype(np.int64)
    B = x.shape[0]

    # single upper-triangular 0/1 mask for diagonal blocks: m[p,c] = c >= p
    dmask = (np.arange(P)[None, :] >= np.arange(P)[:, None]).astype(np.float32)
    ones_row = np.ones((1, T), np.float32)

    per_batch = []
    for b in range(B):
        counts = np.bincount(tok[b], minlength=T).astype(np.float32)
        with np.errstate(divide="ignore"):
            invc = (1.0 / counts).astype(np.float32)
        partial = np.cumsum(invc[tok[b]], dtype=np.float32)
        invf = (1.0 / (10000.0 ** (np.arange(0, 64, 2, dtype=np.float32) / 64.0))
                ).astype(np.float32)
        ang = partial[:, None].astype(np.float32) * invf[None, :]
        cos32 = np.cos(ang).T.astype(np.float32)
        sin32 = np.sin(ang).T.astype(np.float32)
        cos128 = np.ascontiguousarray(np.tile(cos32, (4, 1)))
        # dest-indexed: rope = q*cos + qsh*nsin, qsh[dst] = q[rot_src(dst)]
        nsin128 = np.ascontiguousarray(
            np.concatenate([-sin32, sin32, -sin32, sin32], axis=0))
        pmg = np.take_along_axis(pm[b], tok[b], axis=0).astype(np.float32)
        pmbin = (pmg != 0).astype(np.float32)
        vscale = (np.exp(cs[b]).astype(np.float32) * pmbin).astype(np.float32)
        per_batch.append({
            "xt": np.ascontiguousarray(x[b].T),
            "cos2": cos128,
            "nsin": nsin128,
            "csrow": np.ascontiguousarray(cs[b][None, :]),
            "vscale": np.ascontiguousarray(vscale.reshape(NT, P).T),
            "onec": np.ascontiguousarray(pmbin.reshape(NT, P).T),
        })

    in_maps = []
    for core in range(8):
        b = core // 4
        g = core % 4
        qc = slice(g * 256, (g + 1) * 256)
        kc = slice(C + g * 256, C + (g + 1) * 256)
        vc = slice(2 * C + g * 256, 2 * C + (g + 1) * 256)
        wqk = np.ascontiguousarray(np.concatenate([Wa[:, qc], Wa[:, kc]], axis=1))
        bqk_flat = np.concatenate([ba[qc], ba[kc]])          # [512]
        brot = bqk_flat.reshape(8, 2, 32)[:, ::-1, :].reshape(512)  # rotate halves per 64
        bqk = np.ascontiguousarray(
            np.concatenate([bqk_flat.reshape(4, P).T,
                            brot.reshape(4, P).T], axis=1))
        in_maps.append({
            **per_batch[b],
            "wqk": wqk,
            "wv": np.ascontiguousarray(Wa[:, vc]),
            "wp": np.ascontiguousarray(Wp[g * 256:(g + 1) * 256, :]),
            "bqk": bqk,
            "bv": np.ascontiguousarray(ba[vc][None, :]),
            "dmask": dmask,
            "ones": ones_row,
        })
    return in_maps


def kernel(x, cumulative_scores, padding_mask, W_attn, b_attn, W_proj, b_proj,
           token_index, _results_hook=None):
    nc = build_nc()
    in_maps = make_in_maps(x, cumulative_scores, padding_mask, W_attn, b_attn,
                           W_proj, b_proj, token_index)
    res = run_bass_kernel_spmd(nc, in_maps, list(range(8)))
    if _results_hook is not None:
        _results_hook(res)
    bp = np.asarray(b_proj, dtype=np.float32)
    B = np.asarray(x).shape[0]
    out = np.zeros((B, T, C), np.float32)
    for b in range(B):
        acc = np.zeros((T, C), np.float32)
        for g in range(4):
            acc += res.results[b * 4 + g]["outp"]
        out[b] = acc + bp[None, :]
    return out
